# revision 5
# baseline (speedup 1.0000x reference)
"""MultiHeadLatentAttention Trainium2 kernel (8 NeuronCores, SPMD).

Sharding: core c -> (batch b = c // 4, latent group g = c % 4).
Each core owns query heads [4g, 4g+4) and latent head g for its batch:
  - projects q / k_latent / v_latent for its slice (contraction over H=2048)
  - restores k, v from the latent space
  - causal attention for 4 heads in transposed layout (scoresT[s_k, s_q]),
    softmax denominators via an all-ones matmul, masking via 0/1 probs mask
  - partial o_proj (its 512 input dims -> full 2048 output dims)
Host sums the 4 partial o_proj outputs per batch and adds o_b.

All matmuls run in bf16 with fp32 PSUM accumulation.
"""

import math

import numpy as np
import ml_dtypes

B, S, H = 2, 2048, 2048
NH, HD = 16, 128
NLH, LD = 4, 512
HPC = 4            # query heads per core
NCORES = 8
SQC = 512          # s_q chunk
NQC = S // SQC     # 4 chunks
NKT = H // 128     # 16 contraction tiles for the projections
NSB = S // 128     # 16 s_k blocks
BF16 = ml_dtypes.bfloat16

_CACHE = {}


def _build_program():
    import concourse.bacc as bacc
    import concourse.bass as bass
    import concourse.tile as tile
    from concourse import mybir
    from contextlib import ExitStack

    dt = mybir.dt
    AF = mybir.ActivationFunctionType

    nc = bacc.Bacc("TRN2", target_bir_lowering=False, debug=False,
                   num_devices=NCORES)

    xT = nc.declare_dram_parameter("xT", [H, S], dt.bfloat16, isOutput=False)
    qw = nc.declare_dram_parameter("qwT", [H, HPC * HD], dt.bfloat16, isOutput=False)
    klw = nc.declare_dram_parameter("klwT", [H, LD], dt.bfloat16, isOutput=False)
    vlw = nc.declare_dram_parameter("vlwT", [H, LD], dt.bfloat16, isOutput=False)
    krw = nc.declare_dram_parameter("krwT", [LD, HD], dt.bfloat16, isOutput=False)
    vrw = nc.declare_dram_parameter("vrwT", [LD, HD], dt.bfloat16, isOutput=False)
    ow = nc.declare_dram_parameter("owT", [HPC * HD, H], dt.bfloat16, isOutput=False)
    qb = nc.declare_dram_parameter("qb", [HPC * HD], dt.float32, isOutput=False)
    klb = nc.declare_dram_parameter("klb", [LD], dt.float32, isOutput=False)
    vlb = nc.declare_dram_parameter("vlb", [LD], dt.float32, isOutput=False)
    krb = nc.declare_dram_parameter("krb", [HD], dt.float32, isOutput=False)
    vrb = nc.declare_dram_parameter("vrb", [HD], dt.float32, isOutput=False)
    tri = nc.declare_dram_parameter("tri", [128, 128], dt.bfloat16, isOutput=False)
    outp = nc.declare_dram_parameter("out", [H, S], dt.float32, isOutput=True)

    with tile.TileContext(nc) as tc, ExitStack() as ctx:
        const = ctx.enter_context(tc.tile_pool(name="const", bufs=1))
        xpool = ctx.enter_context(tc.tile_pool(name="xpool", bufs=24))
        lat = ctx.enter_context(tc.tile_pool(name="lat", bufs=10))
        probs_pool = ctx.enter_context(tc.tile_pool(name="probs", bufs=18))
        attn_pool = ctx.enter_context(tc.tile_pool(name="attn", bufs=8))
        small = ctx.enter_context(tc.tile_pool(name="small", bufs=8))
        psum = ctx.enter_context(tc.tile_pool(name="psum", bufs=8, space="PSUM"))

        # ---------------- constants / weights ----------------
        qw_sb = const.tile([128, NKT, HPC * HD], dt.bfloat16, tag="qw")
        nc.sync.dma_start(out=qw_sb, in_=qw.ap().rearrange("(k p) m -> p k m", p=128))
        klw_sb = const.tile([128, NKT, LD], dt.bfloat16, tag="klw")
        nc.sync.dma_start(out=klw_sb, in_=klw.ap().rearrange("(k p) m -> p k m", p=128))
        vlw_sb = const.tile([128, NKT, LD], dt.bfloat16, tag="vlw")
        nc.sync.dma_start(out=vlw_sb, in_=vlw.ap().rearrange("(k p) m -> p k m", p=128))
        krw_sb = const.tile([128, 4, HD], dt.bfloat16, tag="krw")
        nc.sync.dma_start(out=krw_sb, in_=krw.ap().rearrange("(k p) m -> p k m", p=128))
        vrw_sb = const.tile([128, 4, HD], dt.bfloat16, tag="vrw")
        nc.sync.dma_start(out=vrw_sb, in_=vrw.ap().rearrange("(k p) m -> p k m", p=128))
        ow_sb = const.tile([128, 4, H], dt.bfloat16, tag="ow")
        nc.sync.dma_start(out=ow_sb, in_=ow.ap().rearrange("(k p) m -> p k m", p=128))

        qb_sb = const.tile([128, HPC], dt.float32, tag="qb")
        nc.sync.dma_start(out=qb_sb, in_=qb.ap().rearrange("(m p) -> p m", p=128))
        klb_sb = const.tile([128, 4], dt.float32, tag="klb")
        nc.sync.dma_start(out=klb_sb, in_=klb.ap().rearrange("(m p) -> p m", p=128))
        vlb_sb = const.tile([128, 4], dt.float32, tag="vlb")
        nc.sync.dma_start(out=vlb_sb, in_=vlb.ap().rearrange("(m p) -> p m", p=128))
        krb_sb = const.tile([128, 1], dt.float32, tag="krb")
        nc.sync.dma_start(out=krb_sb, in_=krb.ap().rearrange("(m p) -> p m", p=128))

        vrb_ap = vrb.ap()
        vrb_bc = const.tile([128, HD], dt.float32, tag="vrbb")
        nc.sync.dma_start(
            out=vrb_bc,
            in_=bass.AP(tensor=vrb_ap.tensor, offset=vrb_ap.offset,
                        ap=[[0, 128]] + list(vrb_ap.ap)),
        )
        tri_sb = const.tile([128, 128], dt.bfloat16, tag="tri")
        nc.sync.dma_start(out=tri_sb, in_=tri.ap())
        ones_sb = const.tile([128, 128], dt.bfloat16, tag="ones")
        nc.vector.memset(ones_sb, 1.0)

        # persistent activations
        qT_sb = [const.tile([128, S], dt.bfloat16, tag=f"qT{h}", name=f"qT{h}")
                 for h in range(HPC)]
        kT_sb = const.tile([128, S], dt.bfloat16, tag="kT")
        v_sb = const.tile([128, NSB, HD], dt.bfloat16, tag="v")

        # ---------------- phase 1: projections ----------------
        for n in range(NQC):
            xs = []
            for k in range(NKT):
                xt = xpool.tile([128, SQC], dt.bfloat16)
                nc.sync.dma_start(
                    out=xt,
                    in_=xT.ap()[128 * k:128 * (k + 1), SQC * n:SQC * (n + 1)])
                xs.append(xt)

            for h in range(HPC):
                ps = psum.tile([128, SQC], dt.float32, tag="bank")
                for k in range(NKT):
                    nc.tensor.matmul(ps, lhsT=qw_sb[:, k, 128 * h:128 * (h + 1)],
                                     rhs=xs[k], start=(k == 0), stop=(k == NKT - 1))
                nc.scalar.activation(out=qT_sb[h][:, SQC * n:SQC * (n + 1)], in_=ps,
                                     func=AF.Identity, bias=qb_sb[:, h:h + 1])

            klts, vlts = [], []
            for wsb, bsb, dst in ((klw_sb, klb_sb, klts), (vlw_sb, vlb_sb, vlts)):
                for m in range(4):
                    ps = psum.tile([128, SQC], dt.float32, tag="bank")
                    for k in range(NKT):
                        nc.tensor.matmul(ps, lhsT=wsb[:, k, 128 * m:128 * (m + 1)],
                                         rhs=xs[k], start=(k == 0), stop=(k == NKT - 1))
                    t = lat.tile([128, SQC], dt.bfloat16)
                    nc.scalar.activation(out=t, in_=ps, func=AF.Identity,
                                         bias=bsb[:, m:m + 1])
                    dst.append(t)

            # k restore: kT[:, n-chunk] = kr_w @ k_lat^T
            ps = psum.tile([128, SQC], dt.float32, tag="bank")
            for kk in range(4):
                nc.tensor.matmul(ps, lhsT=krw_sb[:, kk, :], rhs=klts[kk],
                                 start=(kk == 0), stop=(kk == 3))
            nc.scalar.activation(out=kT_sb[:, SQC * n:SQC * (n + 1)], in_=ps,
                                 func=AF.Identity, bias=krb_sb[:, 0:1])

            # v restore in natural [s, hd] layout
            for jj in range(4):
                ps = psum.tile([128, SQC], dt.float32, tag="bank")
                for kk in range(4):
                    nc.tensor.matmul(ps[:, :HD],
                                     lhsT=vlts[kk][:, 128 * jj:128 * (jj + 1)],
                                     rhs=vrw_sb[:, kk, :],
                                     start=(kk == 0), stop=(kk == 3))
                nc.vector.tensor_add(out=v_sb[:, 4 * n + jj, :], in0=ps[:, :HD],
                                     in1=vrb_bc)

        # ---------------- phase 2+3: attention + o_proj ----------------
        for iq in range(NQC):
            attn_tiles = []
            for h in range(HPC):
                J = 4 * iq + 4
                ps_av = psum.tile([128, SQC], dt.float32, tag="bank")
                ps_sum = psum.tile([128, SQC], dt.float32, tag="bank")

                def emit_av(j, pt):
                    nc.tensor.matmul(ps_av, lhsT=v_sb[:, j, :], rhs=pt,
                                     start=(j == 0), stop=(j == J - 1))
                    nc.tensor.matmul(ps_sum, lhsT=ones_sb, rhs=pt,
                                     start=(j == 0), stop=(j == J - 1))

                pending = None
                for j in range(J):
                    ps_s = psum.tile([128, SQC], dt.float32, tag="bank")
                    nc.tensor.matmul(ps_s, lhsT=kT_sb[:, 128 * j:128 * (j + 1)],
                                     rhs=qT_sb[h][:, SQC * iq:SQC * (iq + 1)],
                                     start=True, stop=True)
                    pt = probs_pool.tile([128, SQC], dt.bfloat16)
                    d = j - 4 * iq
                    if d < 0:
                        nc.scalar.activation(out=pt, in_=ps_s, func=AF.Exp)
                    else:
                        if d > 0:
                            nc.vector.memset(pt[:, :128 * d], 0.0)
                        nc.scalar.activation(out=pt[:, 128 * d:],
                                             in_=ps_s[:, 128 * d:], func=AF.Exp)
                        nc.vector.tensor_mul(out=pt[:, 128 * d:128 * (d + 1)],
                                             in0=pt[:, 128 * d:128 * (d + 1)],
                                             in1=tri_sb)
                    if pending is not None:
                        emit_av(*pending)
                    pending = (j, pt)
                emit_av(*pending)

                recip = small.tile([128, SQC], dt.float32)
                nc.vector.reciprocal_approx_fast(out=recip, in_=ps_sum)
                at = attn_pool.tile([128, SQC], dt.bfloat16)
                nc.vector.tensor_mul(out=at, in0=ps_av, in1=recip)
                attn_tiles.append(at)

            for m in range(NSB):
                ps_o = psum.tile([128, SQC], dt.float32, tag="bank")
                for h in range(HPC):
                    nc.tensor.matmul(ps_o, lhsT=ow_sb[:, h, 128 * m:128 * (m + 1)],
                                     rhs=attn_tiles[h], start=(h == 0), stop=(h == 3))
                o_sb = small.tile([128, SQC], dt.float32, tag="osb")
                nc.vector.tensor_copy(out=o_sb, in_=ps_o)
                nc.sync.dma_start(
                    out=outp.ap()[128 * m:128 * (m + 1), SQC * iq:SQC * (iq + 1)],
                    in_=o_sb)

    nc.compile()
    return nc


def _get_nc():
    if "nc" not in _CACHE:
        _CACHE["nc"] = _build_program()
    return _CACHE["nc"]


def _make_in_maps(hidden_states, attention_mask, q_w, q_b, kl_w, kl_b, vl_w, vl_b,
                  kr_w, kr_b, vr_w, vr_b, o_w):
    scale = 1.0 / math.sqrt(HD)
    tri01 = (np.asarray(attention_mask[0, 0, :128, :128]) == 0).T.astype(BF16)
    krwT = np.ascontiguousarray(np.asarray(kr_w, np.float32).T).astype(BF16)
    vrwT = np.ascontiguousarray(np.asarray(vr_w, np.float32).T).astype(BF16)
    krb_f = np.asarray(kr_b, np.float32)
    vrb_f = np.asarray(vr_b, np.float32)
    in_maps = []
    for c in range(NCORES):
        b, g = divmod(c, NLH)
        sl = slice(LD * g, LD * (g + 1))
        xTc = np.ascontiguousarray(np.asarray(hidden_states[b], np.float32).T
                                   ).astype(BF16)
        in_maps.append({
            "xT": xTc,
            "qwT": np.ascontiguousarray(
                (np.asarray(q_w[sl], np.float32) * scale).T).astype(BF16),
            "klwT": np.ascontiguousarray(np.asarray(kl_w[sl], np.float32).T
                                         ).astype(BF16),
            "vlwT": np.ascontiguousarray(np.asarray(vl_w[sl], np.float32).T
                                         ).astype(BF16),
            "krwT": krwT,
            "vrwT": vrwT,
            "owT": np.ascontiguousarray(np.asarray(o_w, np.float32)[:, sl].T
                                        ).astype(BF16),
            "qb": (np.asarray(q_b[sl], np.float32) * scale),
            "klb": np.asarray(kl_b[sl], np.float32),
            "vlb": np.asarray(vl_b[sl], np.float32),
            "krb": krb_f,
            "vrb": vrb_f,
            "tri": tri01,
        })
    return in_maps


def _gather(results, o_b):
    o_b = np.asarray(o_b, np.float32)
    outs = []
    for b in range(B):
        acc = np.zeros((H, S), np.float32)
        for g in range(NLH):
            acc += results[b * NLH + g]["out"]
        outs.append(acc.T + o_b[None, :])
    return np.stack(outs).astype(np.float32)


def kernel(hidden_states, position_ids, attention_mask, q_w, q_b, kl_w, kl_b,
           vl_w, vl_b, kr_w, kr_b, vr_w, vr_b, o_w, o_b):
    from concourse.bass_utils import run_bass_kernel_spmd

    nc = _get_nc()
    in_maps = _make_in_maps(hidden_states, attention_mask, q_w, q_b, kl_w, kl_b,
                            vl_w, vl_b, kr_w, kr_b, vr_w, vr_b, o_w)
    res = run_bass_kernel_spmd(nc, in_maps, core_ids=list(range(NCORES)))
    return _gather(res.results, o_b)
